# revision 15
# baseline (speedup 1.0000x reference)
"""Trainium2 Bass kernel for nn_Actor (blended-MoE actor network).

Computation per batch row b:
    c     = softmax(gate(x_b))                          # [4] blend coeffs
    h1    = relu(sum_e c_e (x_b @ W1_e + b1_e))         # [256]
    h2    = relu(sum_e c_e (h1  @ W2_e + b2_e))         # [128]
    mu    = sum_e c_e (h2 @ Wmu_e + bmu_e)              # [17]

Strategy (pure data-parallel over 8 NeuronCores, 16384 rows/core,
supertiles of 512 batch rows):
  * Feature-on-partition layout: activations are [feat, batch] tiles, so
    expert weights load directly as matmul lhsT.  x is transposed + cast
    to bf16 on the host (host prep is outside the timed NEFF).
  * Simplex trick: sum_e c_e A_e = A_3 + sum_{e<3} c_e (A_e - A_3);
    the per-expert input scaling for all (e,k) chunks is ONE DVE op
    through stride-0 APs; biases are blended via K=128 zero-padded
    matmuls so relu needs no bias.
  * Head repack: ONE matmul computes all four expert heads M-packed
    ([128,68] lhsT, rows j=i*4+e), a DVE op scales by the broadcast
    coeffs, and ONE [72,17] selection matmul reduces over experts and
    folds in the blended bias (c rows live at partitions 68:72 of the
    scaled tile).  5 matmuls -> 2.
  * Coefficient broadcast [4,N] -> [128,3,N] and [68,N] via a DRAM-bounce
    DMA (engines cannot broadcast across partitions; DMA can, stride-0
    src).  The k axis of the y tiles reuses the [128,3,N] broadcast via a
    stride-0 AP (halves the broadcast DMA vs materializing both chunks).
  * ELU via exact identity elu(z)+1 = relu(z) + min(exp(z), 1); the +1 is
    folded into the next layer's bias on the host.  Gate chains of THREE
    supertiles share PSUM banks at partition bases 0/32/64 so each
    ACT/DVE softmax/elu op covers all three at once.
  * Deep software pipeline: per iteration i the PE stream is
      L1(t=i-4) | head-MM1(t-2) | L2(t-1) | gateL1(i) | gateL2(i-1) |
      logits+denom(i-2) | head-MM2(t-2)
    so every matmul's producers (relu/DVE/gate chains) completed at
    least most of an iteration earlier and the in-order PE never waits;
    big (128,128)-config matmuls are clustered ahead of the small-config
    gate/head tail to keep the LDWEIGHTS pull-ahead alive.
  * bf16 matmuls with fp32 PSUM accumulation (fp8 was evaluated and is
    numerically out of budget: one delta family in e4m3 already costs
    3e-2 rel err vs the 2e-2 gate).
"""

import sys

for _p in ("/opt/trn_rl_repo",):
    if _p not in sys.path:
        sys.path.append(_p)

import ml_dtypes
import numpy as np

import concourse.bass as bass
import concourse.mybir as mybir
import concourse.tile as tile
from concourse import bacc
from concourse.bass_utils import run_bass_kernel_spmd

AF = mybir.ActivationFunctionType
BF16 = mybir.dt.bfloat16
F32 = mybir.dt.float32
BF = ml_dtypes.bfloat16

NCORES = 8
B_FULL = 131072
BS = B_FULL // NCORES  # 16384 rows per core
NB = 512               # batch tile (matmul free dim)
D_IN = 256
L1 = 256
L2 = 128
NA = 17
GH = 32
LEAD = 4


def build_graph(bs: int = BS, num_devices: int = NCORES):
    """Build + compile the per-core Bass graph (same graph on all cores)."""
    nc = bacc.Bacc(
        "TRN2",
        target_bir_lowering=False,
        debug=False,
        enable_asserts=False,
        num_devices=num_devices,
    )
    d = {}

    def din(name, shape, dt):
        d[name] = nc.dram_tensor(name, shape, dt, kind="ExternalInput").ap()

    # all heavy weights partition-major on the host so each loads in ONE DMA
    din("xt", [D_IN, bs], BF16)            # x shard, transposed
    din("w1b", [128, 2, L1], BF16)         # W1[3] as [part, k, m]
    din("w1d", [128, 3, 2, L1], BF16)      # W1[e]-W1[3]
    din("w2b", [128, 2, L2], BF16)
    din("w2d", [128, 3, 2, L2], BF16)
    din("wmuall", [128, 4 * NA], BF16)     # Wmu[e][:, i] at col i*4+e
    din("sel72", [68 + 4, NA], BF16)       # expert-reduce + bias lhsT
    din("gw1", [2, 128, GH], BF16)
    din("gw2x3", [3 * GH, GH], BF16)       # gate W2 replicated at bases 0/32/64
    din("gwox3", [3 * GH, 4], BF16)
    # bias blend weights, zero-padded to K=128 with the live rows at
    # partition 32v (three variants, one per position in the gate triple)
    din("b1f", [128, 3, 2, 128], BF16)
    din("b2f", [128, 3, L2], BF16)
    din("gb1", [3 * GH, 1], F32)           # tiled x3
    din("gb2p", [3 * GH, 1], F32)          # (gb2 - colsum(gW2_bf16)) x3
    din("gbop", [3 * GH, 1], F32)          # gbo' at rows 32v..32v+3
    out = nc.dram_tensor("out", [NA, bs], F32, kind="ExternalOutput").ap()

    with tile.TileContext(nc) as tc:
        _body(tc, out, d, bs)
    nc.compile()
    return nc


def _body(tc, out, d, bs):
    nc = tc.nc
    nt = bs // NB

    with (
        tc.tile_pool(name="consts", bufs=1) as consts,
        tc.tile_pool(name="io", bufs=2) as io,
        tc.tile_pool(name="act", bufs=2) as act,
        tc.tile_pool(name="ps_g", bufs=1, space="PSUM") as ps_g,
        tc.tile_pool(name="ps_h1", bufs=2, space="PSUM") as ps_h1,
        tc.tile_pool(name="ps_h2", bufs=2, space="PSUM") as ps_h2,
        tc.tile_pool(name="dram", bufs=4, space="DRAM") as dram,
    ):
        # ---- constants/weights ----
        w1b_sb = consts.tile([128, 2, L1], BF16)
        w2b_sb = consts.tile([128, 2, L2], BF16)
        w1d_sb = consts.tile([128, 3, 2, L1], BF16)
        w2d_sb = consts.tile([128, 3, 2, L2], BF16)
        wmuall_sb = consts.tile([128, 4 * NA], BF16)
        sel72_sb = consts.tile([68 + 4, NA], BF16)
        b1f_sb = consts.tile([128, 3, 2, 128], BF16)
        b2f_sb = consts.tile([128, 3, L2], BF16)

        def load_heavy_consts():
            # ordered by first use; alternate queues so the loads are not
            # issue-serialized on one engine
            q = [nc.sync, nc.gpsimd]
            n = 0

            def dma(**kw):
                nonlocal n
                q[n % 2].dma_start(**kw)
                n += 1

            dma(out=w1b_sb, in_=d["w1b"])
            dma(out=w1d_sb, in_=d["w1d"])
            dma(out=b1f_sb, in_=d["b1f"])
            dma(out=w2b_sb, in_=d["w2b"])
            dma(out=w2d_sb, in_=d["w2d"])
            dma(out=b2f_sb, in_=d["b2f"])
            dma(out=wmuall_sb, in_=d["wmuall"])
            dma(out=sel72_sb, in_=d["sel72"])

        gw1_sb = consts.tile([128, 2, GH], BF16)
        for k in range(2):
            nc.gpsimd.dma_start(out=gw1_sb[:, k, :], in_=d["gw1"][k])
        gw2x3_sb = consts.tile([3 * GH, GH], BF16)
        nc.gpsimd.dma_start(out=gw2x3_sb, in_=d["gw2x3"])
        gwox3_sb = consts.tile([3 * GH, 4], BF16)
        nc.gpsimd.dma_start(out=gwox3_sb, in_=d["gwox3"])

        gb1_sb = consts.tile([3 * GH, 1], F32)
        nc.gpsimd.dma_start(out=gb1_sb, in_=d["gb1"])
        gb2p_sb = consts.tile([3 * GH, 1], F32)
        nc.gpsimd.dma_start(out=gb2p_sb, in_=d["gb2p"])
        gbop_sb = consts.tile([3 * GH, 1], F32)
        nc.gpsimd.dma_start(out=gbop_sb, in_=d["gbop"])

        ones96 = consts.tile([3 * GH, 1], BF16)
        nc.vector.memset(ones96, 1.0)

        # persistent zero-padded coefficient slots: c(t) occupies rows
        # 32(t%3)..+3 of column t%9; all other rows stay zero.
        c_slots = consts.tile([128, 9, NB], BF16)
        nc.vector.memset(c_slots, 0.0)

        GA = ps_g.tile([128, NB], F32, tag="ga", name="GA")
        GB = ps_g.tile([128, NB], F32, tag="gb", name="GB")

        state = {}

        def triple_tiles(t0):
            return [t for t in (t0, t0 + 1, t0 + 2) if t < nt]

        def prefetch_xt(t):
            n0 = t * NB
            xt = io.tile([128, 2, NB], BF16, tag="xt", name=f"xt_{t}", bufs=9)
            nc.sync.dma_start(out=xt[:, 0, :], in_=d["xt"][0:128, n0 : n0 + NB])
            nc.sync.dma_start(out=xt[:, 1, :], in_=d["xt"][128:256, n0 : n0 + NB])
            state[("xt", t)] = xt

        def gate_stage1(t0):
            """Gate layer-1 matmuls for the triple + elu chain."""
            ts_ = triple_tiles(t0)
            P = GH * len(ts_)
            for j, t in enumerate(ts_):
                pg1 = GA[GH * j : GH * (j + 1)]
                xt = state[("xt", t)]
                nc.tensor.matmul(pg1, lhsT=gw1_sb[:, 0, :], rhs=xt[:, 0, :], start=True, stop=False)
                nc.tensor.matmul(pg1, lhsT=gw1_sb[:, 1, :], rhs=xt[:, 1, :], start=False, stop=True)
            eg1 = act.tile([P, NB], BF16, tag="eg1", name=f"eg1_{t0}")
            nc.scalar.activation(eg1, GA[0:P], AF.Exp, bias=gb1_sb[0:P])
            rg1 = act.tile([P, NB], BF16, tag="rg1", name=f"rg1_{t0}")
            nc.scalar.activation(rg1, GA[0:P], AF.Relu, bias=gb1_sb[0:P])
            g1 = act.tile([P, NB], BF16, tag="g1", name=f"g1_{t0}")
            nc.vector.scalar_tensor_tensor(
                g1, eg1, 1.0, rg1, op0=mybir.AluOpType.min, op1=mybir.AluOpType.add
            )
            state[("g1", t0)] = g1

        def gate_stage2(t0):
            """Gate layer-2 matmuls (K=32 row groups) + elu chain."""
            ts_ = triple_tiles(t0)
            nm = len(ts_)
            P = GH * nm
            g1 = state.pop(("g1", t0))
            for j in range(nm):
                s = slice(GH * j, GH * (j + 1))
                nc.tensor.matmul(GB[s], lhsT=gw2x3_sb[s, :], rhs=g1[s, :], start=True, stop=True)
            eg2 = act.tile([P, NB], BF16, tag="eg2", name=f"eg2_{t0}")
            nc.scalar.activation(eg2, GB[0:P], AF.Exp, bias=gb2p_sb[0:P])
            rg2 = act.tile([P, NB], BF16, tag="rg2", name=f"rg2_{t0}")
            nc.scalar.activation(rg2, GB[0:P], AF.Relu, bias=gb2p_sb[0:P])
            g2 = act.tile([P, NB], BF16, tag="g2", name=f"g2_{t0}")
            nc.vector.scalar_tensor_tensor(
                g2, eg2, 1.0, rg2, op0=mybir.AluOpType.min, op1=mybir.AluOpType.add
            )
            state[("g2", t0)] = g2

        def gate_stage3a(t0):
            """Logits, exp, softmax denominators + reciprocals for the triple."""
            ts_ = triple_tiles(t0)
            g2 = state.pop(("g2", t0))
            expv = act.tile([GH * 2 + 4, NB], BF16, tag="expv", name=f"expv_{t0}")
            for j, t in enumerate(ts_):
                s4 = slice(GH * j, GH * j + 4)
                nc.tensor.matmul(
                    GA[s4], lhsT=gwox3_sb[GH * j : GH * (j + 1), :],
                    rhs=g2[GH * j : GH * (j + 1), :], start=True, stop=True,
                )
                nc.scalar.activation(expv[s4], GA[s4], AF.Exp, bias=gbop_sb[s4])
            # denominators MUST be read by reciprocal_approx_fast at partition
            # base 0 (nonzero bases give wrong results on HW): j0 -> GB[0:1],
            # j1 -> GA[0:1] (row 0 free once expv j0 read it), j2 -> GB[0:1]
            # again after recip j0 drained it (WAR-ordered).
            den_rows = [GB[0:1], GA[0:1], GB[0:1]]
            rdens = []

            def den_and_recip(j, t):
                s4 = slice(GH * j, GH * j + 4)
                nc.tensor.matmul(
                    den_rows[j], lhsT=ones96[s4], rhs=expv[s4], start=True, stop=True
                )
                rden = act.tile([1, NB], F32, tag=f"rden{j}", name=f"rden_{t}")
                nc.vector.reciprocal_approx_fast(out=rden, in_=den_rows[j])
                rden_dram = dram.tile([1, NB], F32, tag="rden_dram", name=f"rdram_{t}")
                nc.sync.dma_start(out=rden_dram, in_=rden)
                rdens.append(rden_dram)

            for j, t in enumerate(ts_[:2]):
                den_and_recip(j, t)
            if len(ts_) > 2:
                den_and_recip(2, ts_[2])
            state[("expv", t0)] = (expv, rdens)

        def c_bounce(t):
            """Coefficients + DRAM-bounce broadcasts for ONE tile (spread
            across iterations so the gpsimd DMA-issue queue stays uniform)."""
            t0 = 3 * (t // 3)
            j = t - t0
            expv, rdens = state[("expv", t0)]
            if t == min(t0 + 2, nt - 1):
                del state[("expv", t0)]
            s4 = slice(GH * j, GH * j + 4)
            r4sb = act.tile([GH * 2 + 4, NB], F32, tag="r4sb", name=f"r4sb_{t}", bufs=3)
            nc.sync.dma_start(out=r4sb[s4], in_=rdens[j].to_broadcast([4, NB]))
            cs = c_slots[:, t % 9, :]
            nc.vector.tensor_mul(cs[s4], expv[s4], r4sb[s4])

            # bounce all 4 coeff rows; broadcast to [128,3,NB] (deltas)
            # and [68,NB] (head, rows j=i*4+e)
            c4_dram = dram.tile([1, 4, NB], BF16, tag="c4_dram", name=f"c4d_{t}", bufs=6)
            nc.gpsimd.dma_start(out=c4_dram[0], in_=cs[GH * j : GH * j + 4, :])
            cb = io.tile([128, 3, NB], BF16, tag="cb", name=f"cb_{t}", bufs=9)
            nc.gpsimd.dma_start(out=cb, in_=c4_dram[:, 0:3, :].to_broadcast([128, 3, NB]))
            cb68 = io.tile([4 * NA, NB], BF16, tag="cb68", name=f"cb68_{t}", bufs=7)
            nc.gpsimd.dma_start(
                out=cb68, in_=c4_dram.to_broadcast([NA, 4, NB])
            )
            state[("c", t)] = (cs, cb, cb68, c4_dram)

        def cb_k2(cb):
            """[128,3,NB] coeff broadcast viewed as [128,3,2,NB] (stride-0 k)."""
            ap = [list(p) for p in cb.ap]
            return bass.AP(
                tensor=cb.tensor, offset=cb.offset,
                ap=[ap[0], ap[1], [0, 2], ap[2]],
            )

        def dve_y1(t):
            """y1[e,k] = c_e * x_k for all (e,k) in ONE DVE op."""
            xt = state[("xt", t)]
            cs, cb, cb68, c4_dram = state[("c", t)]
            xt3 = bass.AP(
                tensor=xt.tensor, offset=xt.offset,
                ap=[list(xt.ap[0]), [0, 3]] + [list(p) for p in xt.ap[1:]],
            )
            y1 = io.tile([128, 3, 2, NB], BF16, tag="y1", name=f"y1_{t}", bufs=3)
            nc.vector.tensor_mul(y1, xt3, cb_k2(cb))
            state[("y1", t)] = y1

        def heavy_a(t):
            """Layer 1 matmuls + split relu."""
            xt = state.pop(("xt", t))
            cs, cb, cb68, c4_dram = state[("c", t)]
            y1 = state.pop(("y1", t))
            v = t % 3
            ph1 = ps_h1.tile([128, 2, NB], F32, tag="h1", name=f"ph1_{t}")
            h1 = act.tile([128, 2, NB], BF16, tag="h1", name=f"h1_{t}")
            for m in range(2):
                pm = ph1[:, m, :]
                ms = slice(m * 128, (m + 1) * 128)
                nc.tensor.matmul(pm, lhsT=b1f_sb[:, v, m, :], rhs=cs, start=True, stop=False)
                nc.tensor.matmul(pm, lhsT=w1b_sb[:, 0, ms], rhs=xt[:, 0, :], start=False, stop=False)
                nc.tensor.matmul(pm, lhsT=w1b_sb[:, 1, ms], rhs=xt[:, 1, :], start=False, stop=False)
                for e in range(3):
                    for k in range(2):
                        nc.tensor.matmul(
                            pm, lhsT=w1d_sb[:, e, k, ms], rhs=y1[:, e, k, :],
                            start=False, stop=(e == 2 and k == 1),
                        )
                # per-m relu so h1[:,m] is ready while the other half runs
                nc.scalar.activation(h1[:, m, :], ph1[:, m, :], AF.Relu)
            state[("h1", t)] = h1

        def dve_y2(t):
            cs, cb, cb68, c4_dram = state[("c", t)]
            h1 = state[("h1", t)]
            h13 = bass.AP(
                tensor=h1.tensor, offset=h1.offset,
                ap=[list(h1.ap[0]), [0, 3]] + [list(p) for p in h1.ap[1:]],
            )
            y2 = act.tile([128, 3, 2, NB], BF16, tag="y2", name=f"y2_{t}")
            nc.vector.tensor_mul(y2, h13, cb_k2(cb))
            state[("y2", t)] = y2

        def heavy_b(t):
            """Layer 2."""
            cs, cb, cb68, c4_dram = state[("c", t)]
            h1 = state.pop(("h1", t))
            y2 = state.pop(("y2", t))
            v = t % 3
            ph2 = ps_h2.tile([L2, NB], F32, tag="h2", name=f"ph2_{t}")
            nc.tensor.matmul(ph2, lhsT=b2f_sb[:, v, :], rhs=cs, start=True, stop=False)
            nc.tensor.matmul(ph2, lhsT=w2b_sb[:, 0, :], rhs=h1[:, 0, :], start=False, stop=False)
            nc.tensor.matmul(ph2, lhsT=w2b_sb[:, 1, :], rhs=h1[:, 1, :], start=False, stop=False)
            for e in range(3):
                for k in range(2):
                    nc.tensor.matmul(
                        ph2, lhsT=w2d_sb[:, e, k, :], rhs=y2[:, e, k, :],
                        start=False, stop=(e == 2 and k == 1),
                    )
            h2 = act.tile([L2, NB], BF16, tag="h2s", name=f"h2_{t}")
            nc.scalar.activation(h2, ph2, AF.Relu)
            state[("h2", t)] = (ph2, h2)

        def head_mm1(t):
            """All-expert head outputs M-packed into one matmul."""
            ph2, h2 = state.pop(("h2", t))
            cs, cb, cb68, c4_dram = state.pop(("c", t))
            pall = ph2[0 : 4 * NA, :]  # reuse the bank after relu2 read it
            nc.tensor.matmul(pall, lhsT=wmuall_sb, rhs=h2, start=True, stop=True)
            # scaled[j] = c_{j%4} * pall[j]; c rows appended at 68:72 by DMA
            sc = act.tile([68 + 4, NB], BF16, tag="sc", name=f"sc_{t}")
            nc.sync.dma_start(out=sc[68:72, :], in_=c4_dram[0])
            nc.vector.tensor_mul(sc[0:68, :], pall, cb68)
            state[("head", t)] = (ph2, sc)

        def head_mm2(t):
            """Expert reduce + blended bias, then emit the output tile."""
            n0 = t * NB
            ph2, sc = state.pop(("head", t))
            pmu = ph2[64 : 64 + NA, :]
            nc.tensor.matmul(pmu, lhsT=sel72_sb, rhs=sc, start=True, stop=True)
            mu = act.tile([NA, NB], F32, tag="mu", name=f"mu_{t}")
            nc.scalar.copy(mu, pmu)
            nc.sync.dma_start(out=out[:, n0 : n0 + NB], in_=mu)

        # ---- the pipeline ----
        for t in range(min(3, nt)):
            prefetch_xt(t)
        for i in range(nt + LEAD + 2):
            t_a = i - LEAD        # layer-1 tile
            t_b = i - LEAD - 1    # layer-2 tile
            t_h = i - LEAD - 2    # head tile
            if 0 <= t_h < nt:
                head_mm1(t_h)     # first: pall ready early for the DVE scale
            if 0 <= t_b < nt:
                dve_y2(t_b)
            if 0 <= i - 3 < nt:
                c_bounce(i - 3)
            if 0 <= t_a < nt:
                heavy_a(t_a)
            if i == 1:
                load_heavy_consts()
            if 0 <= t_b < nt:
                heavy_b(t_b)
            if i % 3 == 0 and i < nt:
                gate_stage1(i)
            if i + 3 < nt:
                prefetch_xt(i + 3)
            if 0 <= i - 1 < nt and (i - 1) % 3 == 0:
                gate_stage2(i - 1)
            if 0 <= i - 2 < nt and (i - 2) % 3 == 0:
                gate_stage3a(i - 2)
            if 0 <= t_h < nt:
                head_mm2(t_h)
            t_y1 = i - LEAD + 1
            if 0 <= t_y1 < nt:
                dve_y1(t_y1)


def _padk3(b):
    """Three zero-padded K=128 variants of bias-blend weights: variant v has
    the 4 live expert rows at partitions 32v..32v+3."""
    b = np.asarray(b, np.float32)  # [..., 4, M]
    shape = list(b.shape)
    shape[-2] = 128
    outs = []
    for v in range(3):
        o = np.zeros(shape, np.float32)
        o[..., 32 * v : 32 * v + 4, :] = b
        outs.append(o)
    return np.stack(outs, 0).astype(BF)  # [3, ..., 128, M]


def _gbop3(g):
    """[96,1] f32 with the 4 output-gate bias values at rows 32v..32v+3."""
    out = np.zeros((3 * GH, 1), np.float32)
    for v in range(3):
        out[32 * v : 32 * v + 4, 0] = g
    return out


def host_prep(inputs, bs=BS, ncores=NCORES):
    """Convert full f32 inputs to per-core in_maps (weights replicated)."""
    f32 = np.float32
    x = np.asarray(inputs["x"], f32)
    W1 = np.asarray(inputs["W1"], f32)
    b1 = np.asarray(inputs["b1"], f32)
    W2 = np.asarray(inputs["W2"], f32)
    b2 = np.asarray(inputs["b2"], f32)
    Wmu = np.asarray(inputs["Wmu"], f32)
    bmu = np.asarray(inputs["bmu"], f32)
    gW1 = np.asarray(inputs["gW1"], f32)
    gb1 = np.asarray(inputs["gb1"], f32)
    gW2 = np.asarray(inputs["gW2"], f32)
    gb2 = np.asarray(inputs["gb2"], f32)
    gWo = np.asarray(inputs["gWo"], f32)
    gbo = np.asarray(inputs["gbo"], f32)

    gw2_bf = gW2.astype(BF)
    gwo_bf = gWo.astype(BF)
    # head: column j = i*4+e of [128, 68]
    wmuall = np.transpose(Wmu, (1, 2, 0)).reshape(128, 4 * NA)
    sel72 = np.zeros((68 + 4, NA), f32)
    for i_ in range(NA):
        for e in range(4):
            sel72[i_ * 4 + e, i_] = 1.0
    sel72[68:72, :] = bmu
    common = {
        # [k, 128, M] -> partition-major [128, k, M] so each is ONE DMA
        "w1b": W1[3].reshape(2, 128, L1).transpose(1, 0, 2).astype(BF),
        "w1d": (W1[:3] - W1[3]).reshape(3, 2, 128, L1).transpose(2, 0, 1, 3).astype(BF),
        "w2b": W2[3].reshape(2, 128, L2).transpose(1, 0, 2).astype(BF),
        "w2d": (W2[:3] - W2[3]).reshape(3, 2, 128, L2).transpose(2, 0, 1, 3).astype(BF),
        "wmuall": wmuall.astype(BF),
        "sel72": sel72.astype(BF),
        "gw1": gW1.reshape(2, 128, GH).astype(BF),
        "gw2x3": np.tile(gw2_bf, (3, 1)),
        "gwox3": np.tile(gwo_bf, (3, 1)),
        # [3, 2, 128, 128] -> [128, 3, 2, 128]; [3, 128, 128] -> [128, 3, 128]
        "b1f": np.ascontiguousarray(
            _padk3(b1.reshape(4, 2, 128).transpose(1, 0, 2)).transpose(2, 0, 1, 3)
        ),
        "b2f": np.ascontiguousarray(_padk3(b2).transpose(1, 0, 2)),
        "gb1": np.tile(gb1, 3).reshape(3 * GH, 1).astype(f32),
        "gb2p": np.tile(gb2 - gw2_bf.astype(f32).sum(0), 3).reshape(3 * GH, 1).astype(f32),
        "gbop": _gbop3(gbo - gwo_bf.astype(f32).sum(0)),
    }
    xs = x.reshape(ncores, bs, D_IN)
    in_maps = []
    for i in range(ncores):
        m = dict(common)
        m["xt"] = xs[i].T.astype(BF)
        in_maps.append(m)
    return in_maps


_NC_CACHE = {}


def _get_nc():
    key = (BS, NCORES)
    if key not in _NC_CACHE:
        _NC_CACHE[key] = build_graph(BS, NCORES)
    return _NC_CACHE[key]


def kernel(**inputs):
    in_maps = host_prep(inputs)
    nc = _get_nc()
    res = run_bass_kernel_spmd(nc, in_maps, core_ids=list(range(NCORES)))
    outs = [m["out"] for m in res.results]  # each [17, BS] f32
    return np.concatenate([np.asarray(o, np.float32).T for o in outs], axis=0)


if __name__ == "__main__":
    # smoke build
    nc = build_graph(1024, 1)
    print("built ok")


# revision 17
# speedup vs baseline: 1.1765x; 1.1765x over previous
"""Trainium2 Bass kernel for nn_Actor (blended-MoE actor network).

Computation per batch row b:
    c     = softmax(gate(x_b))                          # [4] blend coeffs
    h1    = relu(sum_e c_e (x_b @ W1_e + b1_e))         # [256]
    h2    = relu(sum_e c_e (h1  @ W2_e + b2_e))         # [128]
    mu    = sum_e c_e (h2 @ Wmu_e + bmu_e)              # [17]

Strategy (pure data-parallel over 8 NeuronCores, 16384 rows/core,
supertiles of 512 batch rows):
  * Feature-on-partition layout: activations are [feat, batch] tiles, so
    expert weights load directly as matmul lhsT.  x is transposed + cast
    to bf16 on the host (host prep is outside the timed NEFF).
  * Simplex trick: sum_e c_e A_e = A_3 + sum_{e<3} c_e (A_e - A_3);
    the per-expert input scaling for all (e,k) chunks is ONE DVE op
    through stride-0 APs; biases are blended via K=128 zero-padded
    matmuls so relu needs no bias.
  * Head repack: ONE matmul computes all four expert heads M-packed
    ([128,68] lhsT, rows j=i*4+e), a DVE op scales by the broadcast
    coeffs, and ONE [72,17] selection matmul reduces over experts and
    folds in the blended bias (c rows live at partitions 68:72 of the
    scaled tile).  5 matmuls -> 2.
  * Coefficient broadcast [4,N] -> [128,3,N] and [68,N] via a DRAM-bounce
    DMA (engines cannot broadcast across partitions; DMA can, stride-0
    src).  The k axis of the y tiles reuses the [128,3,N] broadcast via a
    stride-0 AP (halves the broadcast DMA vs materializing both chunks).
  * ELU via exact identity elu(z)+1 = relu(z) + min(exp(z), 1); the +1 is
    folded into the next layer's bias on the host.  Gate chains of THREE
    supertiles share PSUM banks at partition bases 0/32/64 so each
    ACT/DVE softmax/elu op covers all three at once.
  * Deep software pipeline: per iteration i the PE stream is
      L1(t=i-4) | head-MM1(t-2) | L2(t-1) | gateL1(i) | gateL2(i-1) |
      logits+denom(i-2) | head-MM2(t-2)
    so every matmul's producers (relu/DVE/gate chains) completed at
    least most of an iteration earlier and the in-order PE never waits;
    big (128,128)-config matmuls are clustered ahead of the small-config
    gate/head tail to keep the LDWEIGHTS pull-ahead alive.
  * bf16 matmuls with fp32 PSUM accumulation (fp8 was evaluated and is
    numerically out of budget: one delta family in e4m3 already costs
    3e-2 rel err vs the 2e-2 gate).
"""

import sys

for _p in ("/opt/trn_rl_repo",):
    if _p not in sys.path:
        sys.path.append(_p)

import ml_dtypes
import numpy as np

import concourse.bass as bass
import concourse.mybir as mybir
import concourse.tile as tile
from concourse import bacc
from concourse.bass_utils import run_bass_kernel_spmd

AF = mybir.ActivationFunctionType
BF16 = mybir.dt.bfloat16
F32 = mybir.dt.float32
BF = ml_dtypes.bfloat16

NCORES = 8
B_FULL = 131072
BS = B_FULL // NCORES  # 16384 rows per core
NB = 512               # batch tile (matmul free dim)
D_IN = 256
L1 = 256
L2 = 128
NA = 17
GH = 32
LEAD = 4


def build_graph(bs: int = BS, num_devices: int = NCORES):
    """Build + compile the per-core Bass graph (same graph on all cores)."""
    nc = bacc.Bacc(
        "TRN2",
        target_bir_lowering=False,
        debug=False,
        enable_asserts=False,
        num_devices=num_devices,
    )
    d = {}

    def din(name, shape, dt):
        d[name] = nc.dram_tensor(name, shape, dt, kind="ExternalInput").ap()

    # all heavy weights partition-major on the host so each loads in ONE DMA
    din("xt", [D_IN, bs], BF16)            # x shard, transposed
    din("w1b", [128, 2, L1], BF16)         # W1[3] as [part, k, m]
    din("w1d", [128, 3, 2, L1], BF16)      # W1[e]-W1[3]
    din("w2b", [128, 2, L2], BF16)
    din("w2d", [128, 3, 2, L2], BF16)
    din("wmuall", [128, 4 * NA], BF16)     # Wmu[e][:, i] at col i*4+e
    din("sel72", [68 + 4, NA], BF16)       # expert-reduce + bias lhsT
    din("gw1", [2, 128, GH], BF16)
    din("gw2x3", [3 * GH, GH], BF16)       # gate W2 replicated at bases 0/32/64
    din("gwox3", [3 * GH, 4], BF16)
    # bias blend weights, zero-padded to K=128 with the live rows at
    # partition 32v (three variants, one per position in the gate triple)
    din("b1f", [128, 3, 2, 128], BF16)
    din("b2f", [128, 3, L2], BF16)
    din("gb1", [3 * GH, 1], F32)           # tiled x3
    din("gb2p", [3 * GH, 1], F32)          # (gb2 - colsum(gW2_bf16)) x3
    din("gbop", [3 * GH, 1], F32)          # gbo' at rows 32v..32v+3
    out = nc.dram_tensor("out", [NA, bs], F32, kind="ExternalOutput").ap()

    with tile.TileContext(nc) as tc:
        _body(tc, out, d, bs)
    nc.compile()
    return nc


def _body(tc, out, d, bs):
    nc = tc.nc
    nt = bs // NB

    with (
        tc.tile_pool(name="consts", bufs=1) as consts,
        tc.tile_pool(name="io", bufs=2) as io,
        tc.tile_pool(name="act", bufs=2) as act,
        tc.tile_pool(name="ps_g", bufs=1, space="PSUM") as ps_g,
        tc.tile_pool(name="ps_h1", bufs=2, space="PSUM") as ps_h1,
        tc.tile_pool(name="ps_h2", bufs=2, space="PSUM") as ps_h2,
        tc.tile_pool(name="dram", bufs=4, space="DRAM") as dram,
    ):
        # ---- constants/weights ----
        w1b_sb = consts.tile([128, 2, L1], BF16)
        w2b_sb = consts.tile([128, 2, L2], BF16)
        w1d_sb = consts.tile([128, 3, 2, L1], BF16)
        w2d_sb = consts.tile([128, 3, 2, L2], BF16)
        wmuall_sb = consts.tile([128, 4 * NA], BF16)
        sel72_sb = consts.tile([68 + 4, NA], BF16)
        b1f_sb = consts.tile([128, 3, 2, 128], BF16)
        b2f_sb = consts.tile([128, 3, L2], BF16)

        def load_heavy_consts():
            # ordered by first use; alternate queues so the loads are not
            # issue-serialized on one engine
            q = [nc.sync, nc.gpsimd]
            n = 0

            def dma(**kw):
                nonlocal n
                q[n % 2].dma_start(**kw)
                n += 1

            dma(out=w1b_sb, in_=d["w1b"])
            dma(out=w1d_sb, in_=d["w1d"])
            dma(out=b1f_sb, in_=d["b1f"])
            dma(out=w2b_sb, in_=d["w2b"])
            dma(out=w2d_sb, in_=d["w2d"])
            dma(out=b2f_sb, in_=d["b2f"])
            dma(out=wmuall_sb, in_=d["wmuall"])
            dma(out=sel72_sb, in_=d["sel72"])

        gw1_sb = consts.tile([128, 2, GH], BF16)
        for k in range(2):
            nc.gpsimd.dma_start(out=gw1_sb[:, k, :], in_=d["gw1"][k])
        gw2x3_sb = consts.tile([3 * GH, GH], BF16)
        nc.gpsimd.dma_start(out=gw2x3_sb, in_=d["gw2x3"])
        gwox3_sb = consts.tile([3 * GH, 4], BF16)
        nc.gpsimd.dma_start(out=gwox3_sb, in_=d["gwox3"])

        gb1_sb = consts.tile([3 * GH, 1], F32)
        nc.gpsimd.dma_start(out=gb1_sb, in_=d["gb1"])
        gb2p_sb = consts.tile([3 * GH, 1], F32)
        nc.gpsimd.dma_start(out=gb2p_sb, in_=d["gb2p"])
        gbop_sb = consts.tile([3 * GH, 1], F32)
        nc.gpsimd.dma_start(out=gbop_sb, in_=d["gbop"])

        ones96 = consts.tile([3 * GH, 1], BF16)
        nc.vector.memset(ones96, 1.0)

        # persistent zero-padded coefficient slots: c(t) occupies rows
        # 32(t%3)..+3 of column t%9; all other rows stay zero.
        c_slots = consts.tile([128, 9, NB], BF16)
        nc.vector.memset(c_slots, 0.0)

        GA = ps_g.tile([128, NB], F32, tag="ga", name="GA")
        GB = ps_g.tile([128, NB], F32, tag="gb", name="GB")

        state = {}

        def triple_tiles(t0):
            return [t for t in (t0, t0 + 1, t0 + 2) if t < nt]

        def prefetch_xt(t):
            n0 = t * NB
            xt = io.tile([128, 2, NB], BF16, tag="xt", name=f"xt_{t}", bufs=10)
            nc.sync.dma_start(out=xt[:, 0, :], in_=d["xt"][0:128, n0 : n0 + NB])
            nc.sync.dma_start(out=xt[:, 1, :], in_=d["xt"][128:256, n0 : n0 + NB])
            state[("xt", t)] = xt

        def gate_stage1(t0):
            """Gate layer-1 matmuls for the triple + elu chain."""
            ts_ = triple_tiles(t0)
            P = GH * len(ts_)
            for j, t in enumerate(ts_):
                pg1 = GA[GH * j : GH * (j + 1)]
                xt = state[("xt", t)]
                nc.tensor.matmul(pg1, lhsT=gw1_sb[:, 0, :], rhs=xt[:, 0, :], start=True, stop=False)
                nc.tensor.matmul(pg1, lhsT=gw1_sb[:, 1, :], rhs=xt[:, 1, :], start=False, stop=True)
            eg1 = act.tile([P, NB], BF16, tag="eg1", name=f"eg1_{t0}")
            nc.scalar.activation(eg1, GA[0:P], AF.Exp, bias=gb1_sb[0:P])
            rg1 = act.tile([P, NB], BF16, tag="rg1", name=f"rg1_{t0}")
            nc.scalar.activation(rg1, GA[0:P], AF.Relu, bias=gb1_sb[0:P])
            g1 = act.tile([P, NB], BF16, tag="g1", name=f"g1_{t0}")
            nc.vector.scalar_tensor_tensor(
                g1, eg1, 1.0, rg1, op0=mybir.AluOpType.min, op1=mybir.AluOpType.add
            )
            state[("g1", t0)] = g1

        def gate_stage2(t0):
            """Gate layer-2 matmuls (K=32 row groups) + elu chain."""
            ts_ = triple_tiles(t0)
            nm = len(ts_)
            P = GH * nm
            g1 = state.pop(("g1", t0))
            for j in range(nm):
                s = slice(GH * j, GH * (j + 1))
                nc.tensor.matmul(GB[s], lhsT=gw2x3_sb[s, :], rhs=g1[s, :], start=True, stop=True)
            eg2 = act.tile([P, NB], BF16, tag="eg2", name=f"eg2_{t0}")
            nc.scalar.activation(eg2, GB[0:P], AF.Exp, bias=gb2p_sb[0:P])
            rg2 = act.tile([P, NB], BF16, tag="rg2", name=f"rg2_{t0}")
            nc.scalar.activation(rg2, GB[0:P], AF.Relu, bias=gb2p_sb[0:P])
            g2 = act.tile([P, NB], BF16, tag="g2", name=f"g2_{t0}")
            nc.vector.scalar_tensor_tensor(
                g2, eg2, 1.0, rg2, op0=mybir.AluOpType.min, op1=mybir.AluOpType.add
            )
            state[("g2", t0)] = g2

        def gate_stage3a(t0):
            """Logits, exp, softmax denominators + reciprocals for the triple."""
            ts_ = triple_tiles(t0)
            g2 = state.pop(("g2", t0))
            expv = act.tile([GH * 2 + 4, NB], BF16, tag="expv", name=f"expv_{t0}")
            for j, t in enumerate(ts_):
                s4 = slice(GH * j, GH * j + 4)
                nc.tensor.matmul(
                    GA[s4], lhsT=gwox3_sb[GH * j : GH * (j + 1), :],
                    rhs=g2[GH * j : GH * (j + 1), :], start=True, stop=True,
                )
                nc.scalar.activation(expv[s4], GA[s4], AF.Exp, bias=gbop_sb[s4])
            # denominators MUST be read by reciprocal_approx_fast at partition
            # base 0 (nonzero bases give wrong results on HW): j0 -> GB[0:1],
            # j1 -> GA[0:1] (row 0 free once expv j0 read it), j2 -> GB[0:1]
            # again after recip j0 drained it (WAR-ordered).
            den_rows = [GB[0:1], GA[0:1], GB[0:1]]
            rdens = []

            def den_and_recip(j, t):
                s4 = slice(GH * j, GH * j + 4)
                nc.tensor.matmul(
                    den_rows[j], lhsT=ones96[s4], rhs=expv[s4], start=True, stop=True
                )
                rden = act.tile([1, NB], F32, tag=f"rden{j}", name=f"rden_{t}")
                nc.vector.reciprocal_approx_fast(out=rden, in_=den_rows[j])
                rden_dram = dram.tile([1, NB], F32, tag="rden_dram", name=f"rdram_{t}")
                nc.sync.dma_start(out=rden_dram, in_=rden)
                rdens.append(rden_dram)

            for j, t in enumerate(ts_[:2]):
                den_and_recip(j, t)
            if len(ts_) > 2:
                den_and_recip(2, ts_[2])
            state[("expv", t0)] = (expv, rdens)

        def c_bounce(t):
            """Coefficients + DRAM-bounce broadcasts for ONE tile (spread
            across iterations so the gpsimd DMA-issue queue stays uniform)."""
            t0 = 3 * (t // 3)
            j = t - t0
            expv, rdens = state[("expv", t0)]
            if t == min(t0 + 2, nt - 1):
                del state[("expv", t0)]
            s4 = slice(GH * j, GH * j + 4)
            r4sb = act.tile([GH * 2 + 4, NB], F32, tag="r4sb", name=f"r4sb_{t}", bufs=3)
            nc.sync.dma_start(out=r4sb[s4], in_=rdens[j].to_broadcast([4, NB]))
            cs = c_slots[:, t % 9, :]
            nc.vector.tensor_mul(cs[s4], expv[s4], r4sb[s4])

            # bounce all 4 coeff rows; broadcast to [128,3,NB] (deltas)
            # and [68,NB] (head, rows j=i*4+e)
            c4_dram = dram.tile([1, 4, NB], BF16, tag="c4_dram", name=f"c4d_{t}", bufs=6)
            nc.gpsimd.dma_start(out=c4_dram[0], in_=cs[GH * j : GH * j + 4, :])
            cb = io.tile([128, 3, NB], BF16, tag="cb", name=f"cb_{t}", bufs=9)
            nc.gpsimd.dma_start(out=cb, in_=c4_dram[:, 0:3, :].to_broadcast([128, 3, NB]))
            cb68 = io.tile([4 * NA, NB], BF16, tag="cb68", name=f"cb68_{t}", bufs=7)
            nc.gpsimd.dma_start(
                out=cb68, in_=c4_dram.to_broadcast([NA, 4, NB])
            )
            state[("c", t)] = (cs, cb, cb68, c4_dram)

        def cb_k2(cb):
            """[128,3,NB] coeff broadcast viewed as [128,3,2,NB] (stride-0 k)."""
            ap = [list(p) for p in cb.ap]
            return bass.AP(
                tensor=cb.tensor, offset=cb.offset,
                ap=[ap[0], ap[1], [0, 2], ap[2]],
            )

        def dve_y1(t):
            """y1[e,k] = c_e * x_k for all (e,k) in ONE DVE op."""
            xt = state[("xt", t)]
            cs, cb, cb68, c4_dram = state[("c", t)]
            xt3 = bass.AP(
                tensor=xt.tensor, offset=xt.offset,
                ap=[list(xt.ap[0]), [0, 3]] + [list(p) for p in xt.ap[1:]],
            )
            y1 = io.tile([128, 3, 2, NB], BF16, tag="y1", name=f"y1_{t}", bufs=3)
            nc.vector.tensor_mul(y1, xt3, cb_k2(cb))
            state[("y1", t)] = y1

        def heavy_a(t):
            """Layer 1 matmuls + split relu."""
            xt = state.pop(("xt", t))
            cs, cb, cb68, c4_dram = state[("c", t)]
            y1 = state.pop(("y1", t))
            v = t % 3
            ph1 = ps_h1.tile([128, 2, NB], F32, tag="h1", name=f"ph1_{t}")
            h1 = act.tile([128, 2, NB], BF16, tag="h1", name=f"h1_{t}")
            for m in range(2):
                pm = ph1[:, m, :]
                ms = slice(m * 128, (m + 1) * 128)
                nc.tensor.matmul(pm, lhsT=b1f_sb[:, v, m, :], rhs=cs, start=True, stop=False)
                nc.tensor.matmul(pm, lhsT=w1b_sb[:, 0, ms], rhs=xt[:, 0, :], start=False, stop=False)
                nc.tensor.matmul(pm, lhsT=w1b_sb[:, 1, ms], rhs=xt[:, 1, :], start=False, stop=False)
                for e in range(3):
                    for k in range(2):
                        nc.tensor.matmul(
                            pm, lhsT=w1d_sb[:, e, k, ms], rhs=y1[:, e, k, :],
                            start=False, stop=(e == 2 and k == 1),
                        )
                # per-m relu so h1[:,m] is ready while the other half runs
                nc.scalar.activation(h1[:, m, :], ph1[:, m, :], AF.Relu)
            state[("h1", t)] = h1

        def dve_y2(t):
            cs, cb, cb68, c4_dram = state[("c", t)]
            h1 = state[("h1", t)]
            h13 = bass.AP(
                tensor=h1.tensor, offset=h1.offset,
                ap=[list(h1.ap[0]), [0, 3]] + [list(p) for p in h1.ap[1:]],
            )
            y2 = act.tile([128, 3, 2, NB], BF16, tag="y2", name=f"y2_{t}")
            nc.vector.tensor_mul(y2, h13, cb_k2(cb))
            state[("y2", t)] = y2

        def heavy_b(t):
            """Layer 2."""
            cs, cb, cb68, c4_dram = state[("c", t)]
            h1 = state.pop(("h1", t))
            y2 = state.pop(("y2", t))
            v = t % 3
            ph2 = ps_h2.tile([L2, NB], F32, tag="h2", name=f"ph2_{t}")
            nc.tensor.matmul(ph2, lhsT=b2f_sb[:, v, :], rhs=cs, start=True, stop=False)
            nc.tensor.matmul(ph2, lhsT=w2b_sb[:, 0, :], rhs=h1[:, 0, :], start=False, stop=False)
            nc.tensor.matmul(ph2, lhsT=w2b_sb[:, 1, :], rhs=h1[:, 1, :], start=False, stop=False)
            for e in range(3):
                for k in range(2):
                    nc.tensor.matmul(
                        ph2, lhsT=w2d_sb[:, e, k, :], rhs=y2[:, e, k, :],
                        start=False, stop=(e == 2 and k == 1),
                    )
            h2 = act.tile([L2, NB], BF16, tag="h2s", name=f"h2_{t}")
            nc.scalar.activation(h2, ph2, AF.Relu)
            state[("h2", t)] = (ph2, h2)

        def head_mm1(t):
            """All-expert head outputs M-packed into one matmul."""
            ph2, h2 = state.pop(("h2", t))
            cs, cb, cb68, c4_dram = state.pop(("c", t))
            pall = ph2[0 : 4 * NA, :]  # reuse the bank after relu2 read it
            nc.tensor.matmul(pall, lhsT=wmuall_sb, rhs=h2, start=True, stop=True)
            # scaled[j] = c_{j%4} * pall[j]; c rows appended at 68:72 by DMA
            sc = act.tile([68 + 4, NB], BF16, tag="sc", name=f"sc_{t}")
            nc.sync.dma_start(out=sc[68:72, :], in_=c4_dram[0])
            nc.vector.tensor_mul(sc[0:68, :], pall, cb68)
            state[("head", t)] = (ph2, sc)

        def head_mm2(t):
            """Expert reduce + blended bias, then emit the output tile."""
            n0 = t * NB
            ph2, sc = state.pop(("head", t))
            pmu = ph2[64 : 64 + NA, :]
            nc.tensor.matmul(pmu, lhsT=sel72_sb, rhs=sc, start=True, stop=True)
            mu = act.tile([NA, NB], F32, tag="mu", name=f"mu_{t}")
            nc.scalar.copy(mu, pmu)
            nc.sync.dma_start(out=out[:, n0 : n0 + NB], in_=mu)

        # ---- the pipeline ----
        # gate stages run one iteration earlier than strictly needed so the
        # coefficient bounce for tile t completes during iteration t+2 and
        # y1(t) (at t+3) never waits on the DRAM round-trip.
        for t in range(min(4, nt)):
            prefetch_xt(t)
        gate_stage1(0)
        for i in range(nt + LEAD + 2):
            t_a = i - LEAD        # layer-1 tile
            t_b = i - LEAD - 1    # layer-2 tile
            t_h = i - LEAD - 2    # head tile
            if 0 <= t_b < nt:
                dve_y2(t_b)
            if 0 <= t_h < nt:
                head_mm1(t_h)     # early: pall ready when DVE reaches sc
            if 0 <= i - 2 < nt:
                c_bounce(i - 2)
            if 0 <= t_a < nt:
                heavy_a(t_a)
            if i == 1:
                load_heavy_consts()
            if 0 <= t_b < nt:
                heavy_b(t_b)
            if (i + 1) % 3 == 0 and 0 < i + 1 < nt:
                gate_stage1(i + 1)
            if i + 4 < nt:
                prefetch_xt(i + 4)
            if i % 3 == 0 and i < nt:
                gate_stage2(i)
            if 0 <= i - 1 < nt and (i - 1) % 3 == 0:
                gate_stage3a(i - 1)
            if 0 <= t_h < nt:
                head_mm2(t_h)
            t_y1 = i - 3
            if 0 <= t_y1 < nt:
                dve_y1(t_y1)


def _padk3(b):
    """Three zero-padded K=128 variants of bias-blend weights: variant v has
    the 4 live expert rows at partitions 32v..32v+3."""
    b = np.asarray(b, np.float32)  # [..., 4, M]
    shape = list(b.shape)
    shape[-2] = 128
    outs = []
    for v in range(3):
        o = np.zeros(shape, np.float32)
        o[..., 32 * v : 32 * v + 4, :] = b
        outs.append(o)
    return np.stack(outs, 0).astype(BF)  # [3, ..., 128, M]


def _gbop3(g):
    """[96,1] f32 with the 4 output-gate bias values at rows 32v..32v+3."""
    out = np.zeros((3 * GH, 1), np.float32)
    for v in range(3):
        out[32 * v : 32 * v + 4, 0] = g
    return out


def host_prep(inputs, bs=BS, ncores=NCORES):
    """Convert full f32 inputs to per-core in_maps (weights replicated)."""
    f32 = np.float32
    x = np.asarray(inputs["x"], f32)
    W1 = np.asarray(inputs["W1"], f32)
    b1 = np.asarray(inputs["b1"], f32)
    W2 = np.asarray(inputs["W2"], f32)
    b2 = np.asarray(inputs["b2"], f32)
    Wmu = np.asarray(inputs["Wmu"], f32)
    bmu = np.asarray(inputs["bmu"], f32)
    gW1 = np.asarray(inputs["gW1"], f32)
    gb1 = np.asarray(inputs["gb1"], f32)
    gW2 = np.asarray(inputs["gW2"], f32)
    gb2 = np.asarray(inputs["gb2"], f32)
    gWo = np.asarray(inputs["gWo"], f32)
    gbo = np.asarray(inputs["gbo"], f32)

    gw2_bf = gW2.astype(BF)
    gwo_bf = gWo.astype(BF)
    # head: column j = i*4+e of [128, 68]
    wmuall = np.transpose(Wmu, (1, 2, 0)).reshape(128, 4 * NA)
    sel72 = np.zeros((68 + 4, NA), f32)
    for i_ in range(NA):
        for e in range(4):
            sel72[i_ * 4 + e, i_] = 1.0
    sel72[68:72, :] = bmu
    common = {
        # [k, 128, M] -> partition-major [128, k, M] so each is ONE DMA
        "w1b": W1[3].reshape(2, 128, L1).transpose(1, 0, 2).astype(BF),
        "w1d": (W1[:3] - W1[3]).reshape(3, 2, 128, L1).transpose(2, 0, 1, 3).astype(BF),
        "w2b": W2[3].reshape(2, 128, L2).transpose(1, 0, 2).astype(BF),
        "w2d": (W2[:3] - W2[3]).reshape(3, 2, 128, L2).transpose(2, 0, 1, 3).astype(BF),
        "wmuall": wmuall.astype(BF),
        "sel72": sel72.astype(BF),
        "gw1": gW1.reshape(2, 128, GH).astype(BF),
        "gw2x3": np.tile(gw2_bf, (3, 1)),
        "gwox3": np.tile(gwo_bf, (3, 1)),
        # [3, 2, 128, 128] -> [128, 3, 2, 128]; [3, 128, 128] -> [128, 3, 128]
        "b1f": np.ascontiguousarray(
            _padk3(b1.reshape(4, 2, 128).transpose(1, 0, 2)).transpose(2, 0, 1, 3)
        ),
        "b2f": np.ascontiguousarray(_padk3(b2).transpose(1, 0, 2)),
        "gb1": np.tile(gb1, 3).reshape(3 * GH, 1).astype(f32),
        "gb2p": np.tile(gb2 - gw2_bf.astype(f32).sum(0), 3).reshape(3 * GH, 1).astype(f32),
        "gbop": _gbop3(gbo - gwo_bf.astype(f32).sum(0)),
    }
    xs = x.reshape(ncores, bs, D_IN)
    in_maps = []
    for i in range(ncores):
        m = dict(common)
        m["xt"] = xs[i].T.astype(BF)
        in_maps.append(m)
    return in_maps


_NC_CACHE = {}


def _get_nc():
    key = (BS, NCORES)
    if key not in _NC_CACHE:
        _NC_CACHE[key] = build_graph(BS, NCORES)
    return _NC_CACHE[key]


def kernel(**inputs):
    in_maps = host_prep(inputs)
    nc = _get_nc()
    res = run_bass_kernel_spmd(nc, in_maps, core_ids=list(range(NCORES)))
    outs = [m["out"] for m in res.results]  # each [17, BS] f32
    return np.concatenate([np.asarray(o, np.float32).T for o in outs], axis=0)


if __name__ == "__main__":
    # smoke build
    nc = build_graph(1024, 1)
    print("built ok")
